# revision 2
# baseline (speedup 1.0000x reference)
"""Griffin recurrence Trainium2 kernel, v2.

Sharding: 8 cores = 4 batches x 2 channel-halves (192 channels each).
Matmul in bf16 (full PE rate, halves x DMA). The chunked scan of the
reference is computed directly: per-chunk cumulative decay via a
multiplicative tensor_tensor_scan (no ln/exp), clipped division
u*recip(max(D,1e-10)), cumsum scan, and an incremental cross-chunk scan
chained through AP scan initials. sqrt(1-a^2) is a fitted sum of sigmoids
so the ACT engine never swaps activation tables. Elementwise work is split
across Pool (gpsimd) and DVE; output is written bf16.

HW-legality notes (birverifier/codegen): GPSIMD cannot access PSUM;
scalar_tensor_tensor is DVE-only; TT divide is not a valid HW ALU op.
"""

import sys

sys.path.insert(0, "/opt/trn_rl_repo")

from contextlib import ExitStack

import numpy as np
import ml_dtypes

from concourse import bacc, mybir, tile
from concourse.bass_utils import run_bass_kernel_spmd

f32 = mybir.dt.float32
bf16 = mybir.dt.bfloat16
AF = mybir.ActivationFunctionType
ALU = mybir.AluOpType

D_MODEL = 2048
D_REC = 384
CHUNK = 64
NCORE = 8
CH = 192           # channels per core
SCW = 512          # seq-tile width in the steady state
NK = D_MODEL // 128  # 16 k-tiles
NT = 5             # M-tiles (640 = 5*128 packed W rows)
EPS_LOG = 1e-10

# q(p) = sqrt(1 - sigmoid(p)^2) ~= C0 + sum_i Ci*sigmoid(Ai*p + Bi),
# max abs err 8.8e-4 over p in [-14, 14]. All terms live in the sigmoid
# ACT table, so the kernel never swaps activation tables.
QC0 = 0.697865
QTERMS = (
    (0.302546, -0.537589, 1.749442),
    (-3.0, 0.831839, -0.394654),
    (2.303064, 0.830397, -0.058968),
)

_built = {}


def _emit(tc, nc, xT, wT, db0, db1, out, seq):
    nsc = seq // SCW
    nch = seq // CHUNK        # 64 chunks total

    with ExitStack() as ctx:
        const = ctx.enter_context(tc.tile_pool(name="const", bufs=1))
        sm = ctx.enter_context(tc.tile_pool(name="sm", bufs=1))
        xp = ctx.enter_context(tc.tile_pool(name="xp", bufs=3))
        pp = ctx.enter_context(tc.tile_pool(name="pp", bufs=1, space="PSUM"))
        pv = ctx.enter_context(tc.tile_pool(name="pv", bufs=2, space="PSUM"))
        wk = ctx.enter_context(tc.tile_pool(name="wk", bufs=2))

        # constants. x stream owns the SP queue; W/db go on the ACT queue
        # (k-ascending W so the PE can start immediately).
        zeros = const.tile([128, CHUNK], f32, tag="zeros")
        nc.vector.memset(zeros[:], 0.0)
        ones = const.tile([128, CHUNK], f32, tag="ones")
        nc.vector.memset(ones[:], 1.0)
        # W-scan mask: ones with zeros at chunk starts (segmented cumsum)
        mask = const.tile([128, SCW], f32, tag="mask")
        nc.vector.memset(mask[:], 1.0)
        for c0_ in range(0, SCW, CHUNK):
            nc.vector.memset(mask[:, c0_ : c0_ + 1], 0.0)
        wt = []
        for k in range(NK):
            w = const.tile([128, NT * 128], bf16, tag=f"wt{k}")
            if k == 0:
                # t3 column first: it feeds the very first matmul
                nc.scalar.dma_start(w[:, 384:512], wT[0:128, 384:512])
                nc.scalar.dma_start(w[:, 0:384], wT[0:128, 0:384])
                nc.scalar.dma_start(w[:, 512:640], wT[0:128, 512:640])
            else:
                nc.scalar.dma_start(w[:], wT[k * 128 : (k + 1) * 128, :])
            wt.append(w)
        db0_t = const.tile([128, 1], f32, tag="db0")
        nc.scalar.dma_start(db0_t[:], db0[:])
        db1_t = const.tile([128, 1], f32, tag="db1")
        nc.scalar.dma_start(db1_t[:], db1[:])
        # bias columns for the q sigmoid terms: Ai*db + Bi
        qbA, qbB = [], []
        for idx, (_c, al, be) in enumerate(QTERMS):
            ta = const.tile([128, 1], f32, tag=f"qbA{idx}")
            nc.vector.tensor_scalar(ta[:], db0_t[:], al, be, ALU.mult, ALU.add)
            qbA.append(ta)
            tb = const.tile([64, 1], f32, tag=f"qbB{idx}")
            nc.vector.tensor_scalar(tb[:], db1_t[0:64, :], al, be, ALU.mult, ALU.add)
            qbB.append(tb)

        # cross-chunk chains (persistent, written cpc columns per seq-tile)
        CDa = sm.tile([128, nch], f32, tag="CDa")
        CWa = sm.tile([128, nch], f32, tag="CWa")
        INa = sm.tile([128, nch], f32, tag="INa")
        CDb = sm.tile([64, nch], f32, tag="CDb")
        CWb = sm.tile([64, nch], f32, tag="CWb")
        INb = sm.tile([64, nch], f32, tag="INb")

        # steady-state 512-wide tiles; the last 512 is split into two 256s so
        # the post-matmul drain chain after the final matmul is half as long.
        h = SCW // 2
        tiles = [(i * SCW, SCW) for i in range(nsc - 1)]
        tiles += [((nsc - 1) * SCW, h), ((nsc - 1) * SCW + h, h)]

        for s0, scw in tiles:
            ew = nc.gpsimd
            cpc = scw // CHUNK
            gc0 = s0 // CHUNK

            # all 16 k-tiles of this seq-tile in one DMA (bf16); the first
            # tile is split per-k so the PE can start after ~2.5us
            xall_t = xp.tile([128, NK * SCW], bf16, tag="xall")
            xall = xall_t[:, 0 : NK * scw]
            if s0 == 0:
                # per-k DMAs: each matmul k starts as soon as its slice lands
                for k in range(NK):
                    nc.sync.dma_start(
                        xall[:, k * scw : (k + 1) * scw],
                        xT[k * 128 : (k + 1) * 128, s0 : s0 + scw],
                    )
            else:
                nc.sync.dma_start(
                    xall.rearrange("p (k s) -> p k s", k=NK),
                    xT[:, s0 : s0 + scw].rearrange("(k p) s -> p k s", p=128),
                )
            ps = {}
            for t, pool in ((3, pp), (4, pv), (0, pp), (1, pp), (2, pv)):
                p = pool.tile([128, SCW], f32, tag=f"ps{t}")
                for k in range(NK):
                    nc.tensor.matmul(
                        p[:, 0:scw],
                        wt[k][:, t * 128 : (t + 1) * 128],
                        xall[:, k * scw : (k + 1) * scw],
                        start=(k == 0),
                        stop=(k == NK - 1),
                    )
                ps[t] = p[:, 0:scw]

            def wkt(tag, pg, dt=f32, width=None):
                if width is not None:  # small chain tiles
                    t_ = wk.tile([pg, 8], dt, tag=tag)
                    return t_[:, 0:width]
                t_ = wk.tile([pg, SCW], dt, tag=tag)
                return t_[:, 0:scw]

            # B-group first throughout: its tiles (t3, t4) finish matmul
            # first, so its whole drain chain overlaps the A matmuls and the
            # end-of-kernel tail is only A's short chain.
            ab = wkt("ab", 128)  # [aB; iB] + [db1; 0]
            nc.scalar.activation(ab, ps[3], AF.Sigmoid, bias=db1_t[:])
            iB = wkt("iB", 64)
            nc.sync.dma_start(iB, ab[64:128, :])  # realign iB to rows 0..63
            ivB = wkt("ivB", 64)
            nc.vector.tensor_mul(ivB, iB, ps[4][0:64, :])

            qA = wkt("qA", 128)
            qB = wkt("qB", 64)
            for (name, pg, src, qb, qt) in (
                ("B", 64, ps[3], qbB, qB),
                ("A", 128, ps[0], qbA, qA),
            ):
                sgs = []
                for idx, (_c, al, _b) in enumerate(QTERMS):
                    s = wkt(f"qs{name}{idx}", pg)
                    nc.scalar.activation(
                        s, src[0:pg, :], AF.Sigmoid,
                        bias=qb[idx][0:pg, :], scale=al,
                    )
                    sgs.append(s)
                # scalar_tensor_tensor is DVE-only on HW; build q on Pool
                # with in-place tensor_scalar + tensor_tensor ops
                ew.tensor_scalar(
                    sgs[0], sgs[0], QTERMS[0][0], QC0, ALU.mult, ALU.add
                )
                ew.tensor_scalar(
                    sgs[1], sgs[1], QTERMS[1][0], None, ALU.mult
                )
                ew.tensor_scalar(
                    sgs[2], sgs[2], QTERMS[2][0], None, ALU.mult
                )
                ew.tensor_add(sgs[0], sgs[0], sgs[1])
                ew.tensor_add(qt, sgs[0], sgs[2])

            aA = wkt("aA", 128)
            nc.scalar.activation(aA, ps[0], AF.Sigmoid, bias=db0_t[:])
            iA = wkt("iA", 128)
            nc.scalar.activation(iA, ps[1], AF.Sigmoid)
            ivA = wkt("ivA", 128)
            nc.vector.tensor_mul(ivA, iA, ps[2])

            uB = ivB
            ew.tensor_mul(uB, qB, ivB)
            uA = ivA
            ew.tensor_mul(uA, qA, ivA)

            for name, pg, a_ap, u_t, CD, CW, IN in (
                ("B", 64, ab[0:64, :], uB, CDb, CWb, INb),
                ("A", 128, aA, uA, CDa, CWa, INa),
            ):
                # intra-chunk: D = cumprod(a) with chunk resets via
                # one masked scan: D = (a*m)*D_prev + a*(1-m)
                am = wkt(f"am{name}", pg)
                ew.tensor_mul(am, a_ap, mask[0:pg, 0:scw])
                az = wkt(f"az{name}", pg)
                ew.tensor_tensor(az, a_ap, am, ALU.subtract)
                D = wkt(f"D{name}", pg)
                nc.vector.tensor_tensor_scan(
                    D, am, az, 1.0, ALU.mult, ALU.add
                )
                Dc = wkt(f"Dc{name}", pg)
                ew.tensor_scalar_max(Dc, D, EPS_LOG)
                R = wkt(f"R{name}", pg)
                nc.vector.reciprocal_approx_fast(R, Dc)
                w_ = u_t
                ew.tensor_mul(w_, u_t, R)
                W = wkt(f"W{name}", pg)
                nc.vector.tensor_tensor_scan(
                    W, mask[0:pg, 0:scw], w_, 0.0, ALU.mult, ALU.add
                )

                # incremental cross-chunk scan on this tile's boundaries
                g = slice(gc0, gc0 + cpc)
                bd = D[:, CHUNK - 1 :: CHUNK]
                bW = W[:, CHUNK - 1 :: CHUNK]
                bdc = wkt(f"bdc{name}", pg, width=cpc)
                nc.vector.tensor_scalar_max(bdc, bd, EPS_LOG)
                itb = wkt(f"itb{name}", pg, width=cpc)
                nc.vector.tensor_mul(itb, bd, bW)
                cd_init = 1.0 if s0 == 0 else CD[:, gc0 - 1 : gc0]
                nc.vector.tensor_tensor_scan(
                    CD[:, g], bdc, zeros[0:pg, 0:cpc], cd_init,
                    ALU.mult, ALU.add,
                )
                CDc = wkt(f"CDc{name}", pg, width=cpc)
                nc.vector.tensor_scalar_max(CDc, CD[:, g], EPS_LOG)
                CDr = wkt(f"CDr{name}", pg, width=cpc)
                nc.vector.reciprocal_approx_fast(CDr, CDc)
                tms = wkt(f"tms{name}", pg, width=cpc)
                nc.vector.tensor_mul(tms, itb, CDr)
                cw_init = 0.0 if s0 == 0 else CW[:, gc0 - 1 : gc0]
                nc.vector.tensor_tensor_scan(
                    CW[:, g], ones[0:pg, 0:cpc], tms, cw_init,
                    ALU.mult, ALU.add,
                )
                nc.vector.tensor_mul(IN[:, g], CD[:, g], CW[:, g])

                # combine: state = (W + inc) * D, written bf16
                ob = wkt(f"ob{name}", pg, dt=bf16)
                for c in range(cpc):
                    gc = gc0 + c
                    cs = slice(c * CHUNK, (c + 1) * CHUNK)
                    inc = (
                        zeros[0:pg, 0:1] if gc == 0
                        else IN[:, gc - 1 : gc]
                    )
                    nc.vector.scalar_tensor_tensor(
                        ob[:, cs], W[:, cs], inc, D[:, cs],
                        ALU.add, ALU.mult,
                    )
                orow = 0 if name == "A" else 128
                nc.sync.dma_start(
                    out[orow : orow + pg, s0 : s0 + scw], ob
                )


def _build(seq):
    if seq in _built:
        return _built[seq]
    nc = bacc.Bacc(
        "TRN2", target_bir_lowering=False, debug=False, num_devices=NCORE
    )
    xT = nc.dram_tensor("xT", [D_MODEL, seq], bf16, kind="ExternalInput").ap()
    wT = nc.dram_tensor("wT", [D_MODEL, NT * 128], bf16, kind="ExternalInput").ap()
    db0 = nc.dram_tensor("db0", [128, 1], f32, kind="ExternalInput").ap()
    db1 = nc.dram_tensor("db1", [128, 1], f32, kind="ExternalInput").ap()
    out = nc.dram_tensor("out", [CH, seq], bf16, kind="ExternalOutput").ap()
    with tile.TileContext(nc) as tc:
        _emit(tc, nc, xT, wT, db0, db1, out, seq)
    nc.compile()
    _built[seq] = nc
    return nc


def _pack_w(W, h):
    """Pack this half's W rows into 640 rows of 5 M-tiles.

    t0 = a[0:128], t1 = i[0:128], t2 = v[0:128],
    t3 = [a[128:192]; i[128:192]], t4 = [v[128:192]; zeros]."""
    c0 = h * CH
    z = np.zeros((64, W.shape[1]), np.float32)
    return np.concatenate(
        [
            W[c0 : c0 + 128],
            W[D_REC + c0 : D_REC + c0 + 128],
            W[2 * D_REC + c0 : 2 * D_REC + c0 + 128],
            W[c0 + 128 : c0 + 192],
            W[D_REC + c0 + 128 : D_REC + c0 + 192],
            W[2 * D_REC + c0 + 128 : 2 * D_REC + c0 + 192],
            z,
        ],
        axis=0,
    )


def _in_maps(x, W, db):
    maps = []
    xTs = {}
    for core in range(NCORE):
        b, h = core // 2, core % 2
        if b not in xTs:
            xTs[b] = np.ascontiguousarray(x[b].T).astype(ml_dtypes.bfloat16)
        c0 = h * CH
        wTc = np.ascontiguousarray(_pack_w(W, h).T).astype(ml_dtypes.bfloat16)
        db0v = np.ascontiguousarray(db[c0 : c0 + 128].reshape(128, 1))
        db1v = np.ascontiguousarray(
            np.concatenate([db[c0 + 128 : c0 + 192], np.zeros(64, np.float32)]).reshape(
                128, 1
            )
        )
        maps.append({"xT": xTs[b], "wT": wTc, "db0": db0v, "db1": db1v})
    return maps


def kernel(x, W, decay_bias, _trace=False):
    x = np.asarray(x, np.float32)
    W = np.asarray(W, np.float32)
    db = np.asarray(decay_bias, np.float32)
    B, S, _ = x.shape
    nc = _build(S)
    res = run_bass_kernel_spmd(nc, _in_maps(x, W, db), list(range(NCORE)), trace=_trace)
    outf = np.empty((B, S, D_REC), np.float32)
    for core in range(NCORE):
        b, h = core // 2, core % 2
        outf[b, :, h * CH : (h + 1) * CH] = (
            np.asarray(res.results[core]["out"]).astype(np.float32).T
        )
    if _trace:
        return outf, res
    return outf


# revision 3
# speedup vs baseline: 1.0203x; 1.0203x over previous
"""Griffin recurrence Trainium2 kernel, v2.

Sharding: 8 cores = 4 batches x 2 channel-halves (192 channels each).
Matmul in bf16 (full PE rate, halves x DMA). The chunked scan of the
reference is computed directly: per-chunk cumulative decay via a
multiplicative tensor_tensor_scan (no ln/exp), clipped division
u*recip(max(D,1e-10)), cumsum scan, and an incremental cross-chunk scan
chained through AP scan initials. sqrt(1-a^2) is a fitted sum of sigmoids
so the ACT engine never swaps activation tables. Elementwise work is split
across Pool (gpsimd) and DVE; output is written bf16.

HW-legality notes (birverifier/codegen): GPSIMD cannot access PSUM;
scalar_tensor_tensor is DVE-only; TT divide is not a valid HW ALU op.
"""

import sys

sys.path.insert(0, "/opt/trn_rl_repo")

from contextlib import ExitStack

import numpy as np
import ml_dtypes

from concourse import bacc, mybir, tile
from concourse.bass_utils import run_bass_kernel_spmd

f32 = mybir.dt.float32
bf16 = mybir.dt.bfloat16
AF = mybir.ActivationFunctionType
ALU = mybir.AluOpType

D_MODEL = 2048
D_REC = 384
CHUNK = 64
NCORE = 8
CH = 192           # channels per core
SCW = 512          # seq-tile width in the steady state
NK = D_MODEL // 128  # 16 k-tiles
NT = 5             # M-tiles (640 = 5*128 packed W rows)
EPS_LOG = 1e-10

# q(p) = sqrt(1 - sigmoid(p)^2) ~= C0 + sum_i Ci*sigmoid(Ai*p + Bi),
# max abs err 8.8e-4 over p in [-14, 14]. All terms live in the sigmoid
# ACT table, so the kernel never swaps activation tables.
QC0 = 0.697865
QTERMS = (
    (0.302546, -0.537589, 1.749442),
    (-3.0, 0.831839, -0.394654),
    (2.303064, 0.830397, -0.058968),
)

_built = {}


def _emit(tc, nc, xT, wT, db0, db1, out, seq):
    nsc = seq // SCW
    nch = seq // CHUNK        # 64 chunks total

    with ExitStack() as ctx:
        const = ctx.enter_context(tc.tile_pool(name="const", bufs=1))
        sm = ctx.enter_context(tc.tile_pool(name="sm", bufs=1))
        xp = ctx.enter_context(tc.tile_pool(name="xp", bufs=3))
        pp = ctx.enter_context(tc.tile_pool(name="pp", bufs=1, space="PSUM"))
        pv = ctx.enter_context(tc.tile_pool(name="pv", bufs=2, space="PSUM"))
        wk = ctx.enter_context(tc.tile_pool(name="wk", bufs=2))

        # constants. x stream owns the SP queue; W/db go on the ACT queue
        # (k-ascending W so the PE can start immediately).
        zeros = const.tile([128, CHUNK], f32, tag="zeros")
        nc.vector.memset(zeros[:], 0.0)
        ones = const.tile([128, CHUNK], f32, tag="ones")
        nc.vector.memset(ones[:], 1.0)
        # W-scan mask: ones with zeros at chunk starts (segmented cumsum)
        mask = const.tile([128, SCW], f32, tag="mask")
        nc.vector.memset(mask[:], 1.0)
        for c0_ in range(0, SCW, CHUNK):
            nc.vector.memset(mask[:, c0_ : c0_ + 1], 0.0)
        wt = []
        for k in range(NK):
            w = const.tile([128, NT * 128], bf16, tag=f"wt{k}")
            if k == 0:
                # t3 column first, on the SP queue: the ACT queue opens with
                # a hoisted LoadActFuncSet (1.3us) that would gate the first
                # matmul's weights
                nc.sync.dma_start(w[:, 384:512], wT[0:128, 384:512])
                nc.scalar.dma_start(w[:, 0:384], wT[0:128, 0:384])
                nc.scalar.dma_start(w[:, 512:640], wT[0:128, 512:640])
            else:
                nc.scalar.dma_start(w[:], wT[k * 128 : (k + 1) * 128, :])
            wt.append(w)
        db0_t = const.tile([128, 1], f32, tag="db0")
        nc.scalar.dma_start(db0_t[:], db0[:])
        db1_t = const.tile([128, 1], f32, tag="db1")
        nc.scalar.dma_start(db1_t[:], db1[:])
        # bias columns for the q sigmoid terms: Ai*db + Bi
        qbA, qbB = [], []
        for idx, (_c, al, be) in enumerate(QTERMS):
            ta = const.tile([128, 1], f32, tag=f"qbA{idx}")
            nc.vector.tensor_scalar(ta[:], db0_t[:], al, be, ALU.mult, ALU.add)
            qbA.append(ta)
            tb = const.tile([64, 1], f32, tag=f"qbB{idx}")
            nc.vector.tensor_scalar(tb[:], db1_t[0:64, :], al, be, ALU.mult, ALU.add)
            qbB.append(tb)

        # cross-chunk chains (persistent, written cpc columns per seq-tile)
        CDa = sm.tile([128, nch], f32, tag="CDa")
        CWa = sm.tile([128, nch], f32, tag="CWa")
        INa = sm.tile([128, nch], f32, tag="INa")
        CDb = sm.tile([64, nch], f32, tag="CDb")
        CWb = sm.tile([64, nch], f32, tag="CWb")
        INb = sm.tile([64, nch], f32, tag="INb")

        # steady-state 512-wide tiles; the last 512 is split into two 256s so
        # the post-matmul drain chain after the final matmul is half as long.
        h = SCW // 2
        tiles = [(i * SCW, SCW) for i in range(nsc - 1)]
        tiles += [((nsc - 1) * SCW, h), ((nsc - 1) * SCW + h, h)]

        for s0, scw in tiles:
            ew = nc.gpsimd
            cpc = scw // CHUNK
            gc0 = s0 // CHUNK

            # all 16 k-tiles of this seq-tile in one DMA (bf16); the first
            # tile is split per-k so the PE can start after ~2.5us
            xall_t = xp.tile([128, NK * SCW], bf16, tag="xall")
            xall = xall_t[:, 0 : NK * scw]
            if s0 == 0:
                # per-k DMAs: each matmul k starts as soon as its slice lands
                for k in range(NK):
                    nc.sync.dma_start(
                        xall[:, k * scw : (k + 1) * scw],
                        xT[k * 128 : (k + 1) * 128, s0 : s0 + scw],
                    )
            else:
                nc.sync.dma_start(
                    xall.rearrange("p (k s) -> p k s", k=NK),
                    xT[:, s0 : s0 + scw].rearrange("(k p) s -> p k s", p=128),
                )
            ps = {}
            for t, pool in ((3, pp), (4, pv), (0, pp), (1, pp), (2, pv)):
                p = pool.tile([128, SCW], f32, tag=f"ps{t}")
                for k in range(NK):
                    nc.tensor.matmul(
                        p[:, 0:scw],
                        wt[k][:, t * 128 : (t + 1) * 128],
                        xall[:, k * scw : (k + 1) * scw],
                        start=(k == 0),
                        stop=(k == NK - 1),
                    )
                ps[t] = p[:, 0:scw]

            def wkt(tag, pg, dt=f32, width=None):
                if width is not None:  # small chain tiles
                    t_ = wk.tile([pg, 8], dt, tag=tag)
                    return t_[:, 0:width]
                t_ = wk.tile([pg, SCW], dt, tag=tag)
                return t_[:, 0:scw]

            # B-group first throughout: its tiles (t3, t4) finish matmul
            # first, so its whole drain chain overlaps the A matmuls and the
            # end-of-kernel tail is only A's short chain.
            ab = wkt("ab", 128)  # [aB; iB] + [db1; 0]
            nc.scalar.activation(ab, ps[3], AF.Sigmoid, bias=db1_t[:])
            iB = wkt("iB", 64)
            nc.sync.dma_start(iB, ab[64:128, :])  # realign iB to rows 0..63
            ivB = wkt("ivB", 64)
            nc.vector.tensor_mul(ivB, iB, ps[4][0:64, :])

            qA = wkt("qA", 128)
            qB = wkt("qB", 64)
            for (name, pg, src, qb, qt) in (
                ("B", 64, ps[3], qbB, qB),
                ("A", 128, ps[0], qbA, qA),
            ):
                sgs = []
                for idx, (_c, al, _b) in enumerate(QTERMS):
                    s = wkt(f"qs{name}{idx}", pg)
                    nc.scalar.activation(
                        s, src[0:pg, :], AF.Sigmoid,
                        bias=qb[idx][0:pg, :], scale=al,
                    )
                    sgs.append(s)
                # scalar_tensor_tensor is DVE-only on HW; build q on Pool
                # with in-place tensor_scalar + tensor_tensor ops
                ew.tensor_scalar(
                    sgs[0], sgs[0], QTERMS[0][0], QC0, ALU.mult, ALU.add
                )
                ew.tensor_scalar(
                    sgs[1], sgs[1], QTERMS[1][0], None, ALU.mult
                )
                ew.tensor_scalar(
                    sgs[2], sgs[2], QTERMS[2][0], None, ALU.mult
                )
                ew.tensor_add(sgs[0], sgs[0], sgs[1])
                ew.tensor_add(qt, sgs[0], sgs[2])

            aA = wkt("aA", 128)
            nc.scalar.activation(aA, ps[0], AF.Sigmoid, bias=db0_t[:])
            iA = wkt("iA", 128)
            nc.scalar.activation(iA, ps[1], AF.Sigmoid)
            ivA = wkt("ivA", 128)
            nc.vector.tensor_mul(ivA, iA, ps[2])

            for name, pg, a_ap, q_t, iv_t, CD, CW, IN in (
                ("B", 64, ab[0:64, :], qB, ivB, CDb, CWb, INb),
                ("A", 128, aA, qA, ivA, CDa, CWa, INa),
            ):
                # intra-chunk: D = cumprod(a) with chunk resets via
                # one masked scan: D = (a*m)*D_prev + a*(1-m)
                am = wkt(f"am{name}", pg)
                ew.tensor_mul(am, a_ap, mask[0:pg, 0:scw])
                az = wkt(f"az{name}", pg)
                ew.tensor_tensor(az, a_ap, am, ALU.subtract)
                D = wkt(f"D{name}", pg)
                nc.vector.tensor_tensor_scan(
                    D, am, az, 1.0, ALU.mult, ALU.add
                )
                Dc = wkt(f"Dc{name}", pg)
                ew.tensor_scalar_max(Dc, D, EPS_LOG)
                R = wkt(f"R{name}", pg)
                nc.vector.reciprocal_approx_fast(R, Dc)
                # r2 = q*R is ready before iv (iv needs the last matmul in
                # the drain tile), so w = r2*iv is one op after iv lands
                ew.tensor_mul(R, q_t, R)
                w_ = iv_t
                ew.tensor_mul(w_, R, iv_t)
                W = wkt(f"W{name}", pg)
                nc.vector.tensor_tensor_scan(
                    W, mask[0:pg, 0:scw], w_, 0.0, ALU.mult, ALU.add
                )

                # incremental cross-chunk scan on this tile's boundaries
                g = slice(gc0, gc0 + cpc)
                bd = D[:, CHUNK - 1 :: CHUNK]
                bW = W[:, CHUNK - 1 :: CHUNK]
                bdc = wkt(f"bdc{name}", pg, width=cpc)
                nc.vector.tensor_scalar_max(bdc, bd, EPS_LOG)
                itb = wkt(f"itb{name}", pg, width=cpc)
                nc.vector.tensor_mul(itb, bd, bW)
                cd_init = 1.0 if s0 == 0 else CD[:, gc0 - 1 : gc0]
                nc.vector.tensor_tensor_scan(
                    CD[:, g], bdc, zeros[0:pg, 0:cpc], cd_init,
                    ALU.mult, ALU.add,
                )
                CDc = wkt(f"CDc{name}", pg, width=cpc)
                nc.vector.tensor_scalar_max(CDc, CD[:, g], EPS_LOG)
                CDr = wkt(f"CDr{name}", pg, width=cpc)
                nc.vector.reciprocal_approx_fast(CDr, CDc)
                tms = wkt(f"tms{name}", pg, width=cpc)
                nc.vector.tensor_mul(tms, itb, CDr)
                cw_init = 0.0 if s0 == 0 else CW[:, gc0 - 1 : gc0]
                nc.vector.tensor_tensor_scan(
                    CW[:, g], ones[0:pg, 0:cpc], tms, cw_init,
                    ALU.mult, ALU.add,
                )
                nc.vector.tensor_mul(IN[:, g], CD[:, g], CW[:, g])

                # combine: state = (W + inc) * D, written bf16
                ob = wkt(f"ob{name}", pg, dt=bf16)
                for c in range(cpc):
                    gc = gc0 + c
                    cs = slice(c * CHUNK, (c + 1) * CHUNK)
                    inc = (
                        zeros[0:pg, 0:1] if gc == 0
                        else IN[:, gc - 1 : gc]
                    )
                    nc.vector.scalar_tensor_tensor(
                        ob[:, cs], W[:, cs], inc, D[:, cs],
                        ALU.add, ALU.mult,
                    )
                orow = 0 if name == "A" else 128
                nc.sync.dma_start(
                    out[orow : orow + pg, s0 : s0 + scw], ob
                )


def _build(seq):
    if seq in _built:
        return _built[seq]
    nc = bacc.Bacc(
        "TRN2", target_bir_lowering=False, debug=False, num_devices=NCORE
    )
    xT = nc.dram_tensor("xT", [D_MODEL, seq], bf16, kind="ExternalInput").ap()
    wT = nc.dram_tensor("wT", [D_MODEL, NT * 128], bf16, kind="ExternalInput").ap()
    db0 = nc.dram_tensor("db0", [128, 1], f32, kind="ExternalInput").ap()
    db1 = nc.dram_tensor("db1", [128, 1], f32, kind="ExternalInput").ap()
    out = nc.dram_tensor("out", [CH, seq], bf16, kind="ExternalOutput").ap()
    with tile.TileContext(nc) as tc:
        _emit(tc, nc, xT, wT, db0, db1, out, seq)
    nc.compile()
    _built[seq] = nc
    return nc


def _pack_w(W, h):
    """Pack this half's W rows into 640 rows of 5 M-tiles.

    t0 = a[0:128], t1 = i[0:128], t2 = v[0:128],
    t3 = [a[128:192]; i[128:192]], t4 = [v[128:192]; zeros]."""
    c0 = h * CH
    z = np.zeros((64, W.shape[1]), np.float32)
    return np.concatenate(
        [
            W[c0 : c0 + 128],
            W[D_REC + c0 : D_REC + c0 + 128],
            W[2 * D_REC + c0 : 2 * D_REC + c0 + 128],
            W[c0 + 128 : c0 + 192],
            W[D_REC + c0 + 128 : D_REC + c0 + 192],
            W[2 * D_REC + c0 + 128 : 2 * D_REC + c0 + 192],
            z,
        ],
        axis=0,
    )


def _in_maps(x, W, db):
    maps = []
    xTs = {}
    for core in range(NCORE):
        b, h = core // 2, core % 2
        if b not in xTs:
            xTs[b] = np.ascontiguousarray(x[b].T).astype(ml_dtypes.bfloat16)
        c0 = h * CH
        wTc = np.ascontiguousarray(_pack_w(W, h).T).astype(ml_dtypes.bfloat16)
        db0v = np.ascontiguousarray(db[c0 : c0 + 128].reshape(128, 1))
        db1v = np.ascontiguousarray(
            np.concatenate([db[c0 + 128 : c0 + 192], np.zeros(64, np.float32)]).reshape(
                128, 1
            )
        )
        maps.append({"xT": xTs[b], "wT": wTc, "db0": db0v, "db1": db1v})
    return maps


def kernel(x, W, decay_bias, _trace=False):
    x = np.asarray(x, np.float32)
    W = np.asarray(W, np.float32)
    db = np.asarray(decay_bias, np.float32)
    B, S, _ = x.shape
    nc = _build(S)
    res = run_bass_kernel_spmd(nc, _in_maps(x, W, db), list(range(NCORE)), trace=_trace)
    outf = np.empty((B, S, D_REC), np.float32)
    for core in range(NCORE):
        b, h = core // 2, core % 2
        outf[b, :, h * CH : (h + 1) * CH] = (
            np.asarray(res.results[core]["out"]).astype(np.float32).T
        )
    if _trace:
        return outf, res
    return outf
